# revision 9
# baseline (speedup 1.0000x reference)
"""Hard triplet loss over SoftDTW self-distances — TRN2 Bass kernel.

Algorithm (per core, 16 of the 128 signatures, data-parallel over 8 cores):

1. W production (Tensor+Act engines): the pairwise squared distance
   D[i,j] enters only through w = exp(-D/gamma).  Two accumulating PE
   matmuls compute -D/2 in PSUM: x.x (fp8 e4m3, lhsT == rhs == the
   shipped signal) plus (-sq/2,-1/2).(1,sq) (f32, device-derived), and
   one activation computes w = Exp((-D/2) * 2/gamma).  Shipping x in
   fp8 moves the final loss by only ~1e-7 because sq is derived from
   the same quantized values, so D stays an exact distance matrix (of
   a perturbed signal) and D[i,i]=0 cancels exactly.
   Only a |j-i|<16 band is needed:
   the SoftDTW Gibbs weights decay like exp(-|D|/gamma) ~ 3e-6 per
   off-diagonal step, so the band truncation error is ~e^-200.
2. Band gather: the [i-part, j-free] tiles round-trip through a DRAM
   scratch buffer; the re-read uses a diagonal (stride 161) access
   pattern, landing W in scan-ready [sig-part, (row, delta)] layout.
3. DP (Vector engine): in probability domain P = exp(-R/gamma) the
   SoftDTW recurrence is linear:  P[i,j] = w*(P[i-1,j-1] + P[i-1,j]
   + P[i,j-1]), i.e. per row one pair-sum (tensor_tensor add) and one
   hardware scan (tensor_tensor_scan, state=(up+state)*w). 512 serial
   rows; drains guard the same-engine RAW pipeline hazard.
4. Diagonal extract + host epilogue: R(L) = -gamma*ln(P[L-1,L-1]),
   dists = R/(2L), then the tiny triplet-margin reduction in numpy.
"""
import numpy as np

import concourse.bass as bass
import concourse.mybir as mybir
from concourse.bass_utils import run_bass_kernel_spmd

NG_, NF_, NW_ = 5, 10, 8
STEP = NG_ + NF_ + 1            # 16 signatures per writer
MARGIN = np.float32(1.0)
MODEL_LAMBDA = np.float32(0.01)
GAMMA = np.float32(5.0)

B, N, F = 128, 512, 32
NCORES = 8
S = B // NCORES                 # 16 signatures per core
HB = 16                         # half band width
BW = 2 * HB                     # 32 band slots, delta = j - i + HB
SW = BW + 1                     # stored row width (slot BW is a zero pad)
NB = N // 128                   # 4 row blocks of 128
TW = 128 + BW                   # 160 j-columns produced per row block
K = F + 2                       # augmented feature dim
VW = N + BW                     # 544 padded V columns per signature
PADC = np.float32(50.0)         # pad column makes -D ~ -50*(|x|^2+1) -> w=0


def _build_core_kernel():
    nc = bass.Bass()
    vx = nc.declare_dram_parameter("vx", [F, S * N], mybir.dt.float8e4, isOutput=False)
    sdiag = nc.declare_dram_parameter("sdiag", [S, N], mybir.dt.float32, isOutput=True)
    wdd = nc.dram_tensor("wdd", [S * NB * 128 * TW], mybir.dt.bfloat16)
    # NEFF-embedded constants (no per-call upload): VAsq row0 = ones/PADC
    # complete; row1 = PADC (sq lands later via DMA).  UAsq row1 = -1.
    cva = np.full((2, S * VW), PADC, np.float32)
    ones_pads = np.full((S, VW), PADC, np.float32)
    ones_pads[:, HB:HB + N] = 1.0
    cva[0] = ones_pads.reshape(-1)
    c_vasq = nc.inline_tensor(cva, "c_vasq")
    cua = np.zeros((2, S * N), np.float32)
    cua[1] = -0.5
    c_uasq = nc.inline_tensor(cua, "c_uasq")

    from contextlib import ExitStack
    with ExitStack() as es:
        UAsq = es.enter_context(nc.sbuf_tensor([2, S * N], mybir.dt.float32))
        VAx = es.enter_context(nc.sbuf_tensor([F, S * VW], mybir.dt.float8e4))
        VAsq = es.enter_context(nc.sbuf_tensor([2, S * VW], mybir.dt.float32))
        WT0 = es.enter_context(nc.sbuf_tensor([128, TW], mybir.dt.bfloat16))
        WT1 = es.enter_context(nc.sbuf_tensor([128, TW], mybir.dt.bfloat16))
        WB = es.enter_context(nc.sbuf_tensor([S, N * BW], mybir.dt.bfloat16))
        SR = es.enter_context(nc.sbuf_tensor([S, N * SW], mybir.dt.float32))
        UP = es.enter_context(nc.sbuf_tensor([S, BW], mybir.dt.float32))
        UP0 = es.enter_context(nc.sbuf_tensor([S, BW], mybir.dt.float32))
        SD = es.enter_context(nc.sbuf_tensor([S, N], mybir.dt.float32))
        XQ0 = es.enter_context(nc.sbuf_tensor([F, N], mybir.dt.float32))
        XQ1 = es.enter_context(nc.sbuf_tensor([F, N], mybir.dt.float32))
        STG = es.enter_context(nc.sbuf_tensor([1, N], mybir.dt.float32))
        ONE = es.enter_context(nc.sbuf_tensor([F, 1], mybir.dt.float32))
        P0 = es.enter_context(nc.psum_tensor([128, TW], mybir.dt.float32))
        P1 = es.enter_context(nc.psum_tensor([128, TW], mybir.dt.float32))
        PQ = es.enter_context(nc.psum_tensor([1, N], mybir.dt.float32))
        s_in = es.enter_context(nc.semaphore("s_in"))
        s_ms = es.enter_context(nc.semaphore("s_ms"))
        s_xsq = es.enter_context(nc.semaphore("s_xsq"))
        s_mmq = es.enter_context(nc.semaphore("s_mmq"))
        s_sq = es.enter_context(nc.semaphore("s_sq"))
        s_sqd = es.enter_context(nc.semaphore("s_sqd"))
        s_prep = es.enter_context(nc.semaphore("s_prep"))
        s_mm = es.enter_context(nc.semaphore("s_mm"))
        s_exp = es.enter_context(nc.semaphore("s_exp"))
        s_wr = es.enter_context(nc.semaphore("s_wr"))
        s_band = es.enter_context(nc.semaphore("s_band"))
        s_dp = es.enter_context(nc.semaphore("s_dp"))
        s_ext = es.enter_context(nc.semaphore("s_ext"))
        s_out = es.enter_context(nc.semaphore("s_out"))
        block = es.enter_context(nc.Block())
        WT = (WT0, WT1)
        PP = (P0, P1)

        @block.sync
        def _(sync):
            # unpadded x lands in the non-pad columns of VAx
            sync.dma_start(
                out=bass.AP(VAx, HB, [[S * VW, F], [VW, S], [1, N]]),
                in_=bass.AP(vx, 0, [[S * N, F], [N, S], [1, N]]),
            ).then_inc(s_in, 16)
            sync.dma_start(out=VAsq[:, :], in_=c_vasq[:, :]).then_inc(s_in, 16)
            sync.dma_start(out=UAsq[:, :], in_=c_uasq[:, :]).then_inc(s_in, 16)
            # device-computed sq rows -> VAsq row 1, one sig at a time
            # (engines cannot write partition 1; SBUF->SBUF DMAs can)
            for s in range(S):
                sync.wait_ge(s_sq, 2 * s + 1)
                sync.dma_start(
                    out=VAsq[1:2, s * VW + HB: s * VW + HB + N],
                    in_=STG[:, :],
                ).then_inc(s_sqd, 16)
            for ib in range(NB):
                for s in range(S):
                    k = ib * S + s
                    sync.wait_ge(s_exp, k + 1)
                    # plain contiguous write of the [128, TW] tile
                    sync.dma_start(
                        out=bass.AP(wdd, (s * NB + ib) * 128 * TW,
                                    [[TW, 128], [1, TW]]),
                        in_=WT[k % 2][:, :],
                    ).then_inc(s_wr, 16)
                sync.wait_ge(s_wr, 16 * S * (ib + 1))
                # diagonal band re-read: for (s, p, d):
                #   src elem = (s*NB+ib)*128*TW + p*(TW+1) + d
                sync.dma_start(
                    out=bass.AP(WB, ib * 128 * BW,
                                [[N * BW, S], [BW, 128], [1, BW]]),
                    in_=bass.AP(wdd, ib * 128 * TW,
                                [[NB * 128 * TW, S], [TW + 1, 128], [1, BW]]),
                ).then_inc(s_band, 16)
            sync.wait_ge(s_ext, 1)
            sync.dma_start(out=sdiag[:, :], in_=SD[:, :]).then_inc(s_out, 16)
            sync.wait_ge(s_out, 16)

        @block.tensor
        def _(tensor):
            tensor.wait_ge(s_ms, 3)
            # per-sig sq reduction: PQ[0, :] = sum_d XQ[d, :]
            for s in range(S):
                tensor.wait_ge(s_xsq, s + 1)
                if s >= 1:
                    tensor.wait_ge(s_sq, 2 * s)
                tensor.matmul(PQ[:, :], ONE[:, :], (XQ0 if s % 2 == 0 else XQ1)[:, :],
                              start=True, stop=True).then_inc(s_mmq, 1)
            tensor.wait_ge(s_sqd, 16 * S)
            for ib in range(NB):
                for s in range(S):
                    k = ib * S + s
                    if k >= 2:
                        tensor.wait_ge(s_exp, k - 1)
                    tensor.matmul(
                        PP[k % 2][:, :],
                        VAx[:, s * VW + HB + ib * 128:
                            s * VW + HB + ib * 128 + 128],
                        VAx[:, s * VW + ib * 128: s * VW + ib * 128 + TW],
                        start=True, stop=False,
                    )
                    tensor.matmul(
                        PP[k % 2][:, :],
                        UAsq[:, s * N + ib * 128: s * N + ib * 128 + 128],
                        VAsq[:, s * VW + ib * 128: s * VW + ib * 128 + TW],
                        start=False, stop=True,
                    ).then_inc(s_mm, 1)

        @block.scalar
        def _(scalar):
            # Wait for ALL input DMAs: completions may land out of issue
            # order across queues.
            scalar.wait_ge(s_in, 48)
            # per-sig: square x (f32), then stage sq and -sq/2 rows
            for s in range(S):
                if s >= 2:
                    scalar.wait_ge(s_mmq, s - 1)
                scalar.activation(
                    (XQ0 if s % 2 == 0 else XQ1)[:, :],
                    VAx[0:F, s * VW + HB: s * VW + HB + N],
                    mybir.ActivationFunctionType.Square,
                ).then_inc(s_xsq, 1)
                scalar.wait_ge(s_mmq, s + 1)
                if s >= 1:
                    scalar.wait_ge(s_sqd, 16 * s)
                scalar.copy(STG[:, :], PQ[:, :]).then_inc(s_sq, 1)
                scalar.mul(UAsq[0:1, s * N:(s + 1) * N], PQ[:, :], -0.5
                           ).then_inc(s_sq, 1)
            for k in range(NB * S):
                scalar.wait_ge(s_mm, k + 1)
                if k >= 2:
                    scalar.wait_ge(s_wr, 16 * (k - 1))
                scalar.activation(
                    WT[k % 2][:, :], PP[k % 2][:, :],
                    mybir.ActivationFunctionType.Exp,
                    bias=0.0, scale=float(2.0 / GAMMA),
                ).then_inc(s_exp, 1)
            scalar.wait_ge(s_dp, 1)
            scalar.copy(SD[:, :], bass.AP(SR, HB, [[N * SW, S], [SW, N]])
                        ).then_inc(s_ext, 1)

        @block.vector
        def _(vector):
            vector.memset(bass.AP(VAx, 0, [[S * VW, F], [VW, S], [1, HB]]), 0.0
                          ).then_inc(s_ms, 1)
            vector.memset(bass.AP(VAx, HB + N, [[S * VW, F], [VW, S], [1, HB]]), 0.0
                          ).then_inc(s_ms, 1)
            vector.memset(ONE[:, :], 1.0).then_inc(s_ms, 1)
            vector.memset(SR[:, :], 0.0)
            vector.memset(UP0[:, :], 0.0)
            vector.memset(UP0[:, HB:HB + 1], 1.0)
            vector.drain()
            for ib in range(NB):
                vector.wait_ge(s_band, 16 * (ib + 1))
                for i in range(ib * 128, ib * 128 + 128):
                    if i == 0:
                        vector.tensor_tensor_scan(
                            SR[:, 0:BW], UP0[:, :], WB[:, 0:BW], 0.0,
                            mybir.AluOpType.add, mybir.AluOpType.mult)
                        continue
                    po = (i - 1) * SW
                    vector.drain()
                    vector.tensor_tensor(
                        UP[:, :], SR[:, po:po + BW], SR[:, po + 1:po + BW + 1],
                        mybir.AluOpType.add)
                    vector.drain()
                    vector.tensor_tensor_scan(
                        SR[:, i * SW:i * SW + BW], UP[:, :],
                        WB[:, i * BW:(i + 1) * BW], 0.0,
                        mybir.AluOpType.add, mybir.AluOpType.mult)
            vector.engine_nop().then_inc(s_dp, 1)

    return nc


_NC = None


def _get_nc():
    global _NC
    if _NC is None:
        _NC = _build_core_kernel()
    return _NC


def _prep_inputs(data):
    """Only x ships (unpadded, fp8 e4m3: validated end-to-end loss shift
    ~1e-7); the device derives everything else.  sq is computed on device
    FROM the fp8 values (in f32) so the matmul's D[i,i] cancels exactly."""
    fp8 = mybir.dt.np(mybir.dt.float8e4)
    x = data.reshape(NCORES, S, N, F)
    xT = x.astype(fp8).transpose(0, 3, 1, 2)                      # [C, F, S, N]
    return np.ascontiguousarray(xT.reshape(NCORES, F, S * N))


_DISPATCH = None


def _get_dispatch():
    """Build the sharded jitted executable ONCE (run_bass_via_pjrt re-traces
    per call; this mirrors its multi-core path with a cached jit)."""
    global _DISPATCH
    if _DISPATCH is None:
        import jax
        from jax.sharding import Mesh, PartitionSpec
        from jax.experimental.shard_map import shard_map
        from concourse import bass2jax as b2j

        nc = _get_nc()
        b2j.install_neuronx_cc_hook()
        partition_name = (nc.partition_id_tensor.name
                          if nc.partition_id_tensor else None)
        in_names, out_names, out_avals = [], [], []
        out_shapes = []
        for alloc in nc.m.functions[0].allocations:
            if not isinstance(alloc, mybir.MemoryLocationSet):
                continue
            name = alloc.memorylocations[0].name
            if alloc.kind == "ExternalInput":
                if name != partition_name:
                    in_names.append(name)
            elif alloc.kind == "ExternalOutput":
                out_names.append(name)
                shape = tuple(alloc.tensor_shape)
                dtype = mybir.dt.np(alloc.dtype)
                out_avals.append(jax.core.ShapedArray(shape, dtype))
                out_shapes.append((shape, dtype))
        n_params = len(in_names)
        n_outs = len(out_names)
        bind_names = list(in_names) + list(out_names)
        if partition_name is not None:
            bind_names.append(partition_name)
        donate = tuple(range(n_params, n_params + n_outs))

        def _body(*args):
            operands = list(args)
            if partition_name is not None:
                operands.append(b2j.partition_id_tensor())
            outs = b2j._bass_exec_p.bind(
                *operands,
                out_avals=tuple(out_avals),
                in_names=tuple(bind_names),
                out_names=tuple(out_names),
                lowering_input_output_aliases=(),
                sim_require_finite=True,
                sim_require_nnan=True,
                nc=nc,
            )
            return tuple(outs)

        devices = jax.devices()[:NCORES]
        mesh = Mesh(np.asarray(devices), ("core",))
        in_specs = (PartitionSpec("core"),) * (n_params + n_outs)
        out_specs = (PartitionSpec("core"),) * n_outs
        sharded = jax.jit(
            shard_map(_body, mesh=mesh, in_specs=in_specs,
                      out_specs=out_specs, check_rep=False),
            donate_argnums=donate, keep_unused=True)
        _DISPATCH = (sharded, list(in_names), out_shapes)
    return _DISPATCH


_FAST_OK = True


def _run_device(vax):
    """Fast path: cached jitted shard_map dispatch.  Falls back to the
    stock run_bass_kernel_spmd if the cached-jit internals ever break."""
    global _FAST_OK
    if _FAST_OK:
        try:
            sharded, in_names, out_shapes = _get_dispatch()
            assert in_names == ["vx"], in_names
            concat_in = [np.ascontiguousarray(vax.reshape(NCORES * F, S * N))]
            concat_zeros = [np.zeros((NCORES * shp[0],) + shp[1:], dt)
                            for shp, dt in out_shapes]
            out_arrs = sharded(*concat_in, *concat_zeros)
            return np.asarray(out_arrs[0]).reshape(B, N)
        except Exception:
            _FAST_OK = False
    nc = _get_nc()
    in_maps = [{"vx": np.ascontiguousarray(vax[c])} for c in range(NCORES)]
    res = run_bass_kernel_spmd(nc, in_maps, list(range(NCORES)))
    return np.concatenate([res.results[c]["sdiag"] for c in range(NCORES)], 0)


def kernel(data: np.ndarray, lens: np.ndarray) -> np.ndarray:
    data = np.asarray(data, np.float32)
    lens = np.asarray(lens, np.int32)

    vax = _prep_inputs(data)
    sdiag = _run_device(vax)

    L = np.clip(lens, 1, N).astype(np.int64)
    pll = sdiag[np.arange(B), L - 1]
    R = (-GAMMA * np.log(np.maximum(pll, np.float32(1e-30)))).astype(np.float32)
    dists = (R / (np.float32(2.0) * L.astype(np.float32))).astype(np.float32)

    d = dists.reshape(NW_, STEP)
    dm = ((d[:, :, None] + d[:, None, :]) * np.float32(0.5)).astype(np.float32)
    g = NG_ + 1
    dmg = dm[:, :g, :g]
    neg = dm[:, :g, g:]
    scores = np.maximum(dmg[:, :, :, None] + MARGIN - neg[:, :, None, :],
                        np.float32(0.0))
    maxj = scores.max(axis=(2, 3)).astype(np.float32)
    sum_lks = maxj.sum(axis=1) * np.float32(g * NF_)
    nnz = (maxj != 0).astype(np.float32).sum(axis=1) * np.float32(g * NF_)
    lv = sum_lks / (nnz + np.float32(1.0))
    tril = np.tril(np.ones((g, g), bool), k=-1)
    only_pos = np.where(tril[None], dmg, np.float32(0.0)).sum(axis=(1, 2)) * (
        MODEL_LAMBDA / np.float32(NG_))
    loss = (lv + only_pos).sum() / np.float32(NW_)
    return np.float32(loss)


# revision 11
# speedup vs baseline: 1.1568x; 1.1568x over previous
"""Hard triplet loss over SoftDTW self-distances — TRN2 Bass kernel.

Algorithm (per core, 16 of the 128 signatures, data-parallel over 8 cores):

1. W production (Tensor+Act engines): the pairwise squared distance
   D[i,j] enters only through w = exp(-D/gamma).  Two accumulating PE
   matmuls compute -D/2 in PSUM: x.x (fp8 e4m3, lhsT == rhs == the
   shipped signal) plus (-sq/2,-1/2).(1,sq) (f32, device-derived), and
   one activation computes w = Exp((-D/2) * 2/gamma).  Shipping x in
   fp8 moves the final loss by only ~1e-7 because sq is derived from
   the same quantized values, so D stays an exact distance matrix (of
   a perturbed signal) and D[i,i]=0 cancels exactly.
   Only a |j-i|<16 band is needed:
   the SoftDTW Gibbs weights decay like exp(-|D|/gamma) ~ 3e-6 per
   off-diagonal step, so the band truncation error is ~e^-200.
2. Band gather: the [i-part, j-free] tiles round-trip through a DRAM
   scratch buffer; the re-read uses a diagonal (stride 161) access
   pattern, landing W in scan-ready [sig-part, (row, delta)] layout.
3. DP (Vector engine): in probability domain P = exp(-R/gamma) the
   SoftDTW recurrence is linear:  P[i,j] = w*(P[i-1,j-1] + P[i-1,j]
   + P[i,j-1]), i.e. per row one pair-sum (tensor_tensor add) and one
   hardware scan (tensor_tensor_scan, state=(up+state)*w). 512 serial
   rows; drains guard the same-engine RAW pipeline hazard.
4. Diagonal extract + host epilogue: R(L) = -gamma*ln(P[L-1,L-1]),
   dists = R/(2L), then the tiny triplet-margin reduction in numpy.
"""
import numpy as np

import concourse.bass as bass
import concourse.mybir as mybir
from concourse.bass_utils import run_bass_kernel_spmd

NG_, NF_, NW_ = 5, 10, 8
STEP = NG_ + NF_ + 1            # 16 signatures per writer
MARGIN = np.float32(1.0)
MODEL_LAMBDA = np.float32(0.01)
GAMMA = np.float32(5.0)

B, N, F = 128, 512, 32
NCORES = 8
S = B // NCORES                 # 16 signatures per core
HB = 16                         # half band width
BW = 2 * HB                     # 32 band slots, delta = j - i + HB
SW = BW + 1                     # stored row width (slot BW is a zero pad)
NB = N // 128                   # 4 row blocks of 128
TW = 128 + BW                   # 160 j-columns produced per row block
K = F + 2                       # augmented feature dim
VW = N + BW                     # 544 padded V columns per signature
PADC = np.float32(50.0)         # pad column makes -D ~ -50*(|x|^2+1) -> w=0


def _build_core_kernel():
    nc = bass.Bass()
    vx = nc.declare_dram_parameter("vx", [F, S * N], mybir.dt.float8e4, isOutput=False)
    # output is P-1 in bf16: the loss signal lives in eps = P-1 (~1e-3),
    # where bf16 relative precision (0.4%) is ample; halves transfer bytes
    sdiag = nc.declare_dram_parameter("sdiag", [S, N], mybir.dt.bfloat16, isOutput=True)
    wdd = nc.dram_tensor("wdd", [S * NB * 128 * TW], mybir.dt.bfloat16)
    # NEFF-embedded constants (no per-call upload): VAsq row0 = ones/PADC
    # complete; row1 = PADC (sq lands later via DMA).  UAsq row1 = -1.
    cva = np.full((2, S * VW), PADC, np.float32)
    ones_pads = np.full((S, VW), PADC, np.float32)
    ones_pads[:, HB:HB + N] = 1.0
    cva[0] = ones_pads.reshape(-1)
    c_vasq = nc.inline_tensor(cva, "c_vasq")
    cua = np.zeros((2, S * N), np.float32)
    cua[1] = -0.5
    c_uasq = nc.inline_tensor(cua, "c_uasq")

    from contextlib import ExitStack
    with ExitStack() as es:
        UAsq = es.enter_context(nc.sbuf_tensor([2, S * N], mybir.dt.float32))
        VAx = es.enter_context(nc.sbuf_tensor([F, S * VW], mybir.dt.float8e4))
        VAsq = es.enter_context(nc.sbuf_tensor([2, S * VW], mybir.dt.float32))
        WT0 = es.enter_context(nc.sbuf_tensor([128, TW], mybir.dt.bfloat16))
        WT1 = es.enter_context(nc.sbuf_tensor([128, TW], mybir.dt.bfloat16))
        WB = es.enter_context(nc.sbuf_tensor([S, N * BW], mybir.dt.bfloat16))
        SR = es.enter_context(nc.sbuf_tensor([S, N * SW], mybir.dt.float32))
        UP = es.enter_context(nc.sbuf_tensor([S, BW], mybir.dt.float32))
        UP0 = es.enter_context(nc.sbuf_tensor([S, BW], mybir.dt.float32))
        SD = es.enter_context(nc.sbuf_tensor([S, N], mybir.dt.bfloat16))
        XQ0 = es.enter_context(nc.sbuf_tensor([F, N], mybir.dt.float32))
        XQ1 = es.enter_context(nc.sbuf_tensor([F, N], mybir.dt.float32))
        STG = es.enter_context(nc.sbuf_tensor([1, N], mybir.dt.float32))
        ONE = es.enter_context(nc.sbuf_tensor([F, 1], mybir.dt.float32))
        P0 = es.enter_context(nc.psum_tensor([128, TW], mybir.dt.float32))
        P1 = es.enter_context(nc.psum_tensor([128, TW], mybir.dt.float32))
        PQ = es.enter_context(nc.psum_tensor([1, N], mybir.dt.float32))
        s_in = es.enter_context(nc.semaphore("s_in"))
        s_ms = es.enter_context(nc.semaphore("s_ms"))
        s_xsq = es.enter_context(nc.semaphore("s_xsq"))
        s_mmq = es.enter_context(nc.semaphore("s_mmq"))
        s_sq = es.enter_context(nc.semaphore("s_sq"))
        s_sqd = es.enter_context(nc.semaphore("s_sqd"))
        s_prep = es.enter_context(nc.semaphore("s_prep"))
        s_mm = es.enter_context(nc.semaphore("s_mm"))
        s_exp = es.enter_context(nc.semaphore("s_exp"))
        s_wr = es.enter_context(nc.semaphore("s_wr"))
        s_band = es.enter_context(nc.semaphore("s_band"))
        s_dp = es.enter_context(nc.semaphore("s_dp"))
        s_ext = es.enter_context(nc.semaphore("s_ext"))
        s_out = es.enter_context(nc.semaphore("s_out"))
        block = es.enter_context(nc.Block())
        WT = (WT0, WT1)
        PP = (P0, P1)

        @block.sync
        def _(sync):
            # unpadded x lands in the non-pad columns of VAx
            sync.dma_start(
                out=bass.AP(VAx, HB, [[S * VW, F], [VW, S], [1, N]]),
                in_=bass.AP(vx, 0, [[S * N, F], [N, S], [1, N]]),
            ).then_inc(s_in, 16)
            sync.dma_start(out=VAsq[:, :], in_=c_vasq[:, :]).then_inc(s_in, 16)
            sync.dma_start(out=UAsq[:, :], in_=c_uasq[:, :]).then_inc(s_in, 16)
            # device-computed sq rows -> VAsq row 1, one sig at a time
            # (engines cannot write partition 1; SBUF->SBUF DMAs can)
            for s in range(S):
                sync.wait_ge(s_sq, 2 * s + 1)
                sync.dma_start(
                    out=VAsq[1:2, s * VW + HB: s * VW + HB + N],
                    in_=STG[:, :],
                ).then_inc(s_sqd, 16)
            for ib in range(NB):
                for s in range(S):
                    k = ib * S + s
                    sync.wait_ge(s_exp, k + 1)
                    # plain contiguous write of the [128, TW] tile
                    sync.dma_start(
                        out=bass.AP(wdd, (s * NB + ib) * 128 * TW,
                                    [[TW, 128], [1, TW]]),
                        in_=WT[k % 2][:, :],
                    ).then_inc(s_wr, 16)
                sync.wait_ge(s_wr, 16 * S * (ib + 1))
                # diagonal band re-read: for (s, p, d):
                #   src elem = (s*NB+ib)*128*TW + p*(TW+1) + d
                sync.dma_start(
                    out=bass.AP(WB, ib * 128 * BW,
                                [[N * BW, S], [BW, 128], [1, BW]]),
                    in_=bass.AP(wdd, ib * 128 * TW,
                                [[NB * 128 * TW, S], [TW + 1, 128], [1, BW]]),
                ).then_inc(s_band, 16)
            sync.wait_ge(s_ext, 1)
            sync.dma_start(out=sdiag[:, :], in_=SD[:, :]).then_inc(s_out, 16)
            sync.wait_ge(s_out, 16)

        @block.tensor
        def _(tensor):
            tensor.wait_ge(s_ms, 3)
            # per-sig sq reduction: PQ[0, :] = sum_d XQ[d, :]
            for s in range(S):
                tensor.wait_ge(s_xsq, s + 1)
                if s >= 1:
                    tensor.wait_ge(s_sq, 2 * s)
                tensor.matmul(PQ[:, :], ONE[:, :], (XQ0 if s % 2 == 0 else XQ1)[:, :],
                              start=True, stop=True).then_inc(s_mmq, 1)
            tensor.wait_ge(s_sqd, 16 * S)
            for ib in range(NB):
                for s in range(S):
                    k = ib * S + s
                    if k >= 2:
                        tensor.wait_ge(s_exp, k - 1)
                    tensor.matmul(
                        PP[k % 2][:, :],
                        VAx[:, s * VW + HB + ib * 128:
                            s * VW + HB + ib * 128 + 128],
                        VAx[:, s * VW + ib * 128: s * VW + ib * 128 + TW],
                        start=True, stop=False,
                    )
                    tensor.matmul(
                        PP[k % 2][:, :],
                        UAsq[:, s * N + ib * 128: s * N + ib * 128 + 128],
                        VAsq[:, s * VW + ib * 128: s * VW + ib * 128 + TW],
                        start=False, stop=True,
                    ).then_inc(s_mm, 1)

        @block.scalar
        def _(scalar):
            # Wait for ALL input DMAs: completions may land out of issue
            # order across queues.
            scalar.wait_ge(s_in, 48)
            # per-sig: square x (f32), then stage sq and -sq/2 rows
            for s in range(S):
                if s >= 2:
                    scalar.wait_ge(s_mmq, s - 1)
                scalar.activation(
                    (XQ0 if s % 2 == 0 else XQ1)[:, :],
                    VAx[0:F, s * VW + HB: s * VW + HB + N],
                    mybir.ActivationFunctionType.Square,
                ).then_inc(s_xsq, 1)
                scalar.wait_ge(s_mmq, s + 1)
                if s >= 1:
                    scalar.wait_ge(s_sqd, 16 * s)
                scalar.copy(STG[:, :], PQ[:, :]).then_inc(s_sq, 1)
                scalar.mul(UAsq[0:1, s * N:(s + 1) * N], PQ[:, :], -0.5
                           ).then_inc(s_sq, 1)
            for k in range(NB * S):
                scalar.wait_ge(s_mm, k + 1)
                if k >= 2:
                    scalar.wait_ge(s_wr, 16 * (k - 1))
                scalar.activation(
                    WT[k % 2][:, :], PP[k % 2][:, :],
                    mybir.ActivationFunctionType.Exp,
                    bias=0.0, scale=float(2.0 / GAMMA),
                ).then_inc(s_exp, 1)
            scalar.wait_ge(s_dp, 1)
            scalar.activation(SD[:, :], bass.AP(SR, HB, [[N * SW, S], [SW, N]]),
                              mybir.ActivationFunctionType.Copy,
                              bias=-1.0, scale=1.0).then_inc(s_ext, 1)

        @block.vector
        def _(vector):
            vector.memset(bass.AP(VAx, 0, [[S * VW, F], [VW, S], [1, HB]]), 0.0
                          ).then_inc(s_ms, 1)
            vector.memset(bass.AP(VAx, HB + N, [[S * VW, F], [VW, S], [1, HB]]), 0.0
                          ).then_inc(s_ms, 1)
            vector.memset(ONE[:, :], 1.0).then_inc(s_ms, 1)
            vector.memset(SR[:, :], 0.0)
            vector.memset(UP0[:, :], 0.0)
            vector.memset(UP0[:, HB:HB + 1], 1.0)
            vector.drain()
            for ib in range(NB):
                vector.wait_ge(s_band, 16 * (ib + 1))
                for i in range(ib * 128, ib * 128 + 128):
                    if i == 0:
                        vector.tensor_tensor_scan(
                            SR[:, 0:BW], UP0[:, :], WB[:, 0:BW], 0.0,
                            mybir.AluOpType.add, mybir.AluOpType.mult)
                        continue
                    po = (i - 1) * SW
                    vector.drain()
                    vector.tensor_tensor(
                        UP[:, :], SR[:, po:po + BW], SR[:, po + 1:po + BW + 1],
                        mybir.AluOpType.add)
                    vector.drain()
                    vector.tensor_tensor_scan(
                        SR[:, i * SW:i * SW + BW], UP[:, :],
                        WB[:, i * BW:(i + 1) * BW], 0.0,
                        mybir.AluOpType.add, mybir.AluOpType.mult)
            vector.engine_nop().then_inc(s_dp, 1)

    return nc


_NC = None


def _get_nc():
    global _NC
    if _NC is None:
        _NC = _build_core_kernel()
    return _NC


def _prep_inputs(data):
    """Only x ships (unpadded, fp8 e4m3: validated end-to-end loss shift
    ~1e-7); the device derives everything else.  sq is computed on device
    FROM the fp8 values (in f32) so the matmul's D[i,i] cancels exactly."""
    fp8 = mybir.dt.np(mybir.dt.float8e4)
    x = data.reshape(NCORES, S, N, F)
    xT = x.astype(fp8).transpose(0, 3, 1, 2)                      # [C, F, S, N]
    return np.ascontiguousarray(xT.reshape(NCORES, F, S * N))


_DISPATCH = None


def _get_dispatch():
    """Build the sharded jitted executable ONCE (run_bass_via_pjrt re-traces
    per call; this mirrors its multi-core path with a cached jit)."""
    global _DISPATCH
    if _DISPATCH is None:
        import jax
        from jax.sharding import Mesh, PartitionSpec
        from jax.experimental.shard_map import shard_map
        from concourse import bass2jax as b2j

        nc = _get_nc()
        b2j.install_neuronx_cc_hook()
        partition_name = (nc.partition_id_tensor.name
                          if nc.partition_id_tensor else None)
        in_names, out_names, out_avals = [], [], []
        out_shapes = []
        for alloc in nc.m.functions[0].allocations:
            if not isinstance(alloc, mybir.MemoryLocationSet):
                continue
            name = alloc.memorylocations[0].name
            if alloc.kind == "ExternalInput":
                if name != partition_name:
                    in_names.append(name)
            elif alloc.kind == "ExternalOutput":
                out_names.append(name)
                shape = tuple(alloc.tensor_shape)
                dtype = mybir.dt.np(alloc.dtype)
                out_avals.append(jax.core.ShapedArray(shape, dtype))
                out_shapes.append((shape, dtype))
        n_params = len(in_names)
        n_outs = len(out_names)
        bind_names = list(in_names) + list(out_names)
        if partition_name is not None:
            bind_names.append(partition_name)
        donate = tuple(range(n_params, n_params + n_outs))

        def _body(*args):
            operands = list(args)
            if partition_name is not None:
                operands.append(b2j.partition_id_tensor())
            outs = b2j._bass_exec_p.bind(
                *operands,
                out_avals=tuple(out_avals),
                in_names=tuple(bind_names),
                out_names=tuple(out_names),
                lowering_input_output_aliases=(),
                sim_require_finite=True,
                sim_require_nnan=True,
                nc=nc,
            )
            return tuple(outs)

        devices = jax.devices()[:NCORES]
        mesh = Mesh(np.asarray(devices), ("core",))
        in_specs = (PartitionSpec("core"),) * (n_params + n_outs)
        out_specs = (PartitionSpec("core"),) * n_outs
        sharded = jax.jit(
            shard_map(_body, mesh=mesh, in_specs=in_specs,
                      out_specs=out_specs, check_rep=False),
            donate_argnums=donate, keep_unused=True)
        _DISPATCH = (sharded, list(in_names), out_shapes)
    return _DISPATCH


_FAST_OK = True


def _run_device(vax):
    """Fast path: cached jitted shard_map dispatch.  Falls back to the
    stock run_bass_kernel_spmd if the cached-jit internals ever break."""
    global _FAST_OK
    if _FAST_OK:
        try:
            sharded, in_names, out_shapes = _get_dispatch()
            assert in_names == ["vx"], in_names
            concat_in = [np.ascontiguousarray(vax.reshape(NCORES * F, S * N))]
            concat_zeros = [np.zeros((NCORES * shp[0],) + shp[1:], dt)
                            for shp, dt in out_shapes]
            out_arrs = sharded(*concat_in, *concat_zeros)
            return np.asarray(out_arrs[0]).reshape(B, N)
        except Exception:
            _FAST_OK = False
    nc = _get_nc()
    in_maps = [{"vx": np.ascontiguousarray(vax[c])} for c in range(NCORES)]
    res = run_bass_kernel_spmd(nc, in_maps, list(range(NCORES)))
    return np.concatenate([res.results[c]["sdiag"] for c in range(NCORES)], 0)


def kernel(data: np.ndarray, lens: np.ndarray) -> np.ndarray:
    data = np.asarray(data, np.float32)
    lens = np.asarray(lens, np.int32)

    vax = _prep_inputs(data)
    sdiag = _run_device(vax)

    L = np.clip(lens, 1, N).astype(np.int64)
    pll = np.float32(1.0) + sdiag[np.arange(B), L - 1].astype(np.float32)
    R = (-GAMMA * np.log(np.maximum(pll, np.float32(1e-30)))).astype(np.float32)
    dists = (R / (np.float32(2.0) * L.astype(np.float32))).astype(np.float32)

    d = dists.reshape(NW_, STEP)
    dm = ((d[:, :, None] + d[:, None, :]) * np.float32(0.5)).astype(np.float32)
    g = NG_ + 1
    dmg = dm[:, :g, :g]
    neg = dm[:, :g, g:]
    scores = np.maximum(dmg[:, :, :, None] + MARGIN - neg[:, :, None, :],
                        np.float32(0.0))
    maxj = scores.max(axis=(2, 3)).astype(np.float32)
    sum_lks = maxj.sum(axis=1) * np.float32(g * NF_)
    nnz = (maxj != 0).astype(np.float32).sum(axis=1) * np.float32(g * NF_)
    lv = sum_lks / (nnz + np.float32(1.0))
    tril = np.tril(np.ones((g, g), bool), k=-1)
    only_pos = np.where(tril[None], dmg, np.float32(0.0)).sum(axis=(1, 2)) * (
        MODEL_LAMBDA / np.float32(NG_))
    loss = (lv + only_pos).sum() / np.float32(NW_)
    return np.float32(loss)


# revision 12
# speedup vs baseline: 1.2157x; 1.0509x over previous
"""Hard triplet loss over SoftDTW self-distances — TRN2 Bass kernel.

Algorithm (per core, 16 of the 128 signatures, data-parallel over 8 cores):

1. W production (Tensor+Act engines): the pairwise squared distance
   D[i,j] enters only through w = exp(-D/gamma).  Two accumulating PE
   matmuls compute -D/2 in PSUM: x.x (fp8 e4m3, lhsT == rhs == the
   shipped signal) plus (-sq/2,-1/2).(1,sq) (f32, device-derived), and
   one activation computes w = Exp((-D/2) * 2/gamma).  Shipping x in
   fp8 moves the final loss by only ~1e-7 because sq is derived from
   the same quantized values, so D stays an exact distance matrix (of
   a perturbed signal) and D[i,i]=0 cancels exactly.
   Only a |j-i|<16 band is needed:
   the SoftDTW Gibbs weights decay like exp(-|D|/gamma) ~ 3e-6 per
   off-diagonal step, so the band truncation error is ~e^-200.
2. Band gather: the [i-part, j-free] tiles round-trip through a DRAM
   scratch buffer; the re-read uses a diagonal (stride 161) access
   pattern, landing W in scan-ready [sig-part, (row, delta)] layout.
3. DP (Vector engine): in probability domain P = exp(-R/gamma) the
   SoftDTW recurrence is linear:  P[i,j] = w*(P[i-1,j-1] + P[i-1,j]
   + P[i,j-1]), i.e. per row one pair-sum (tensor_tensor add) and one
   hardware scan (tensor_tensor_scan, state=(up+state)*w). 512 serial
   rows; drains guard the same-engine RAW pipeline hazard.
4. Diagonal extract + host epilogue: R(L) = -gamma*ln(P[L-1,L-1]),
   dists = R/(2L), then the tiny triplet-margin reduction in numpy.
"""
import numpy as np

import concourse.bass as bass
import concourse.mybir as mybir
from concourse.bass_utils import run_bass_kernel_spmd

NG_, NF_, NW_ = 5, 10, 8
STEP = NG_ + NF_ + 1            # 16 signatures per writer
MARGIN = np.float32(1.0)
MODEL_LAMBDA = np.float32(0.01)
GAMMA = np.float32(5.0)

B, N, F = 128, 512, 32
NCORES = 8
S = B // NCORES                 # 16 signatures per core
HB = 16                         # half band width
BW = 2 * HB                     # 32 band slots, delta = j - i + HB
SW = BW + 1                     # stored row width (slot BW is a zero pad)
NB = N // 128                   # 4 row blocks of 128
TW = 128 + BW                   # 160 j-columns produced per row block
K = F + 2                       # augmented feature dim
VW = N + BW                     # 544 padded V columns per signature
PADC = np.float32(50.0)         # pad column makes -D ~ -50*(|x|^2+1) -> w=0


def _build_core_kernel():
    nc = bass.Bass()
    vx = nc.declare_dram_parameter("vx", [F, S * N], mybir.dt.float8e4, isOutput=False)
    # output is P-1 in bf16: the loss signal lives in eps = P-1 (~1e-3),
    # where bf16 relative precision (0.4%) is ample; halves transfer bytes
    sdiag = nc.declare_dram_parameter("sdiag", [S, N], mybir.dt.bfloat16, isOutput=True)
    wdd = nc.dram_tensor("wdd", [S * NB * 128 * TW], mybir.dt.bfloat16)
    # NEFF-embedded constants (no per-call upload): VAsq row0 = ones/PADC
    # complete; row1 = PADC (sq lands later via DMA).  UAsq row1 = -1.
    cva = np.full((2, S * VW), PADC, np.float32)
    ones_pads = np.full((S, VW), PADC, np.float32)
    ones_pads[:, HB:HB + N] = 1.0
    cva[0] = ones_pads.reshape(-1)
    c_vasq = nc.inline_tensor(cva, "c_vasq")
    cua = np.zeros((2, S * N), np.float32)
    cua[1] = -0.5
    c_uasq = nc.inline_tensor(cua, "c_uasq")

    from contextlib import ExitStack
    with ExitStack() as es:
        UAsq = es.enter_context(nc.sbuf_tensor([2, S * N], mybir.dt.float32))
        VAx = es.enter_context(nc.sbuf_tensor([F, S * VW], mybir.dt.float8e4))
        VAsq = es.enter_context(nc.sbuf_tensor([2, S * VW], mybir.dt.float32))
        WT0 = es.enter_context(nc.sbuf_tensor([128, TW], mybir.dt.bfloat16))
        WT1 = es.enter_context(nc.sbuf_tensor([128, TW], mybir.dt.bfloat16))
        WB = es.enter_context(nc.sbuf_tensor([S, N * BW], mybir.dt.bfloat16))
        SR = es.enter_context(nc.sbuf_tensor([S, N * SW], mybir.dt.float32))
        UP = es.enter_context(nc.sbuf_tensor([S, BW], mybir.dt.float32))
        UP0 = es.enter_context(nc.sbuf_tensor([S, BW], mybir.dt.float32))
        SD = es.enter_context(nc.sbuf_tensor([S, N], mybir.dt.bfloat16))
        XQ0 = es.enter_context(nc.sbuf_tensor([F, N], mybir.dt.float32))
        XQ1 = es.enter_context(nc.sbuf_tensor([F, N], mybir.dt.float32))
        STG = es.enter_context(nc.sbuf_tensor([1, N], mybir.dt.float32))
        ONE = es.enter_context(nc.sbuf_tensor([F, 1], mybir.dt.float32))
        P0 = es.enter_context(nc.psum_tensor([128, TW], mybir.dt.float32))
        P1 = es.enter_context(nc.psum_tensor([128, TW], mybir.dt.float32))
        PQ = es.enter_context(nc.psum_tensor([1, N], mybir.dt.float32))
        s_in = es.enter_context(nc.semaphore("s_in"))
        s_ms = es.enter_context(nc.semaphore("s_ms"))
        s_xsq = es.enter_context(nc.semaphore("s_xsq"))
        s_mmq = es.enter_context(nc.semaphore("s_mmq"))
        s_sq = es.enter_context(nc.semaphore("s_sq"))
        s_sqd = es.enter_context(nc.semaphore("s_sqd"))
        s_prep = es.enter_context(nc.semaphore("s_prep"))
        s_mm = es.enter_context(nc.semaphore("s_mm"))
        s_exp = es.enter_context(nc.semaphore("s_exp"))
        s_wr = es.enter_context(nc.semaphore("s_wr"))
        s_band = es.enter_context(nc.semaphore("s_band"))
        s_dp = es.enter_context(nc.semaphore("s_dp"))
        s_ext = es.enter_context(nc.semaphore("s_ext"))
        s_out = es.enter_context(nc.semaphore("s_out"))
        block = es.enter_context(nc.Block())
        WT = (WT0, WT1)
        PP = (P0, P1)

        @block.sync
        def _(sync):
            # unpadded x lands in the non-pad columns of VAx
            sync.dma_start(
                out=bass.AP(VAx, HB, [[S * VW, F], [VW, S], [1, N]]),
                in_=bass.AP(vx, 0, [[S * N, F], [N, S], [1, N]]),
            ).then_inc(s_in, 16)
            sync.dma_start(out=VAsq[:, :], in_=c_vasq[:, :]).then_inc(s_in, 16)
            sync.dma_start(out=UAsq[:, :], in_=c_uasq[:, :]).then_inc(s_in, 16)
            # device-computed sq rows -> VAsq row 1, one sig at a time
            # (engines cannot write partition 1; SBUF->SBUF DMAs can)
            for s in range(S):
                sync.wait_ge(s_sq, 2 * s + 1)
                sync.dma_start(
                    out=VAsq[1:2, s * VW + HB: s * VW + HB + N],
                    in_=STG[:, :],
                ).then_inc(s_sqd, 16)
            for ib in range(NB):
                for s in range(S):
                    k = ib * S + s
                    sync.wait_ge(s_exp, k + 1)
                    # plain contiguous write of the [128, TW] tile
                    sync.dma_start(
                        out=bass.AP(wdd, (s * NB + ib) * 128 * TW,
                                    [[TW, 128], [1, TW]]),
                        in_=WT[k % 2][:, :],
                    ).then_inc(s_wr, 16)
                sync.wait_ge(s_wr, 16 * S * (ib + 1))
                # diagonal band re-read: for (s, p, d):
                #   src elem = (s*NB+ib)*128*TW + p*(TW+1) + d
                sync.dma_start(
                    out=bass.AP(WB, ib * 128 * BW,
                                [[N * BW, S], [BW, 128], [1, BW]]),
                    in_=bass.AP(wdd, ib * 128 * TW,
                                [[NB * 128 * TW, S], [TW + 1, 128], [1, BW]]),
                ).then_inc(s_band, 16)
            sync.wait_ge(s_ext, 1)
            sync.dma_start(out=sdiag[:, :], in_=SD[:, :]).then_inc(s_out, 16)
            sync.wait_ge(s_out, 16)

        @block.tensor
        def _(tensor):
            tensor.wait_ge(s_ms, 3)
            # per-sig sq reduction: PQ[0, :] = sum_d XQ[d, :]
            for s in range(S):
                tensor.wait_ge(s_xsq, s + 1)
                if s >= 1:
                    tensor.wait_ge(s_sq, 2 * s)
                tensor.matmul(PQ[:, :], ONE[:, :], (XQ0 if s % 2 == 0 else XQ1)[:, :],
                              start=True, stop=True).then_inc(s_mmq, 1)
            tensor.wait_ge(s_sqd, 16 * S)
            for ib in range(NB):
                for s in range(S):
                    k = ib * S + s
                    if k >= 2:
                        tensor.wait_ge(s_exp, k - 1)
                    tensor.matmul(
                        PP[k % 2][:, :],
                        VAx[:, s * VW + HB + ib * 128:
                            s * VW + HB + ib * 128 + 128],
                        VAx[:, s * VW + ib * 128: s * VW + ib * 128 + TW],
                        start=True, stop=False,
                    )
                    tensor.matmul(
                        PP[k % 2][:, :],
                        UAsq[:, s * N + ib * 128: s * N + ib * 128 + 128],
                        VAsq[:, s * VW + ib * 128: s * VW + ib * 128 + TW],
                        start=False, stop=True,
                    ).then_inc(s_mm, 1)

        @block.scalar
        def _(scalar):
            # Wait for ALL input DMAs: completions may land out of issue
            # order across queues.
            scalar.wait_ge(s_in, 48)
            # per-sig: square x (f32), then stage sq and -sq/2 rows
            for s in range(S):
                if s >= 2:
                    scalar.wait_ge(s_mmq, s - 1)
                scalar.activation(
                    (XQ0 if s % 2 == 0 else XQ1)[:, :],
                    VAx[0:F, s * VW + HB: s * VW + HB + N],
                    mybir.ActivationFunctionType.Square,
                ).then_inc(s_xsq, 1)
                scalar.wait_ge(s_mmq, s + 1)
                if s >= 1:
                    scalar.wait_ge(s_sqd, 16 * s)
                scalar.copy(STG[:, :], PQ[:, :]).then_inc(s_sq, 1)
                scalar.mul(UAsq[0:1, s * N:(s + 1) * N], PQ[:, :], -0.5
                           ).then_inc(s_sq, 1)
            for k in range(NB * S):
                scalar.wait_ge(s_mm, k + 1)
                if k >= 2:
                    scalar.wait_ge(s_wr, 16 * (k - 1))
                scalar.activation(
                    WT[k % 2][:, :], PP[k % 2][:, :],
                    mybir.ActivationFunctionType.Exp,
                    bias=0.0, scale=float(2.0 / GAMMA),
                ).then_inc(s_exp, 1)
            scalar.wait_ge(s_dp, 1)
            scalar.activation(SD[:, :], bass.AP(SR, HB, [[N * SW, S], [SW, N]]),
                              mybir.ActivationFunctionType.Copy,
                              bias=-1.0, scale=1.0).then_inc(s_ext, 1)

        @block.vector
        def _(vector):
            vector.memset(bass.AP(VAx, 0, [[S * VW, F], [VW, S], [1, HB]]), 0.0
                          ).then_inc(s_ms, 1)
            vector.memset(bass.AP(VAx, HB + N, [[S * VW, F], [VW, S], [1, HB]]), 0.0
                          ).then_inc(s_ms, 1)
            vector.memset(ONE[:, :], 1.0).then_inc(s_ms, 1)
            vector.memset(SR[:, :], 0.0)
            vector.memset(UP0[:, :], 0.0)
            vector.memset(UP0[:, HB:HB + 1], 1.0)
            vector.drain()
            for ib in range(NB):
                vector.wait_ge(s_band, 16 * (ib + 1))
                for i in range(ib * 128, ib * 128 + 128):
                    if i == 0:
                        vector.tensor_tensor_scan(
                            SR[:, 0:BW], UP0[:, :], WB[:, 0:BW], 0.0,
                            mybir.AluOpType.add, mybir.AluOpType.mult)
                        continue
                    po = (i - 1) * SW
                    vector.drain()
                    vector.tensor_tensor(
                        UP[:, :], SR[:, po:po + BW], SR[:, po + 1:po + BW + 1],
                        mybir.AluOpType.add)
                    vector.drain()
                    vector.tensor_tensor_scan(
                        SR[:, i * SW:i * SW + BW], UP[:, :],
                        WB[:, i * BW:(i + 1) * BW], 0.0,
                        mybir.AluOpType.add, mybir.AluOpType.mult)
            vector.engine_nop().then_inc(s_dp, 1)

    return nc


_NC = None


def _get_nc():
    global _NC
    if _NC is None:
        _NC = _build_core_kernel()
    return _NC


_CONV = None


def _prep_inputs(data):
    """Only x ships (unpadded, fp8 e4m3: validated end-to-end loss shift
    ~1e-7); the device derives everything else.  sq is computed on device
    FROM the fp8 values (in f32) so the matmul's D[i,i] cancels exactly.
    XLA-CPU does the transpose+quantize ~3x faster than numpy/ml_dtypes."""
    global _CONV
    if _CONV is None:
        try:
            import jax
            import jax.numpy as jnp
            cpu = jax.devices("cpu")[0]

            @jax.jit
            def _conv(x):
                x = x.reshape(NCORES, S, N, F)
                return (x.transpose(0, 3, 1, 2)
                        .astype(jnp.float8_e4m3fn)
                        .reshape(NCORES, F, S * N))

            def conv(x):
                with jax.default_device(cpu):
                    return np.asarray(_conv(jax.device_put(x, cpu)))
            conv(np.zeros((B, N, F), np.float32))      # validate once
            _CONV = conv
        except Exception:
            fp8 = mybir.dt.np(mybir.dt.float8e4)

            def conv(x):
                xT = x.reshape(NCORES, S, N, F).astype(fp8).transpose(0, 3, 1, 2)
                return np.ascontiguousarray(xT.reshape(NCORES, F, S * N))
            _CONV = conv
    return _CONV(data)


_DISPATCH = None


def _get_dispatch():
    """Build the sharded jitted executable ONCE (run_bass_via_pjrt re-traces
    per call; this mirrors its multi-core path with a cached jit)."""
    global _DISPATCH
    if _DISPATCH is None:
        import jax
        from jax.sharding import Mesh, PartitionSpec
        from jax.experimental.shard_map import shard_map
        from concourse import bass2jax as b2j

        nc = _get_nc()
        b2j.install_neuronx_cc_hook()
        partition_name = (nc.partition_id_tensor.name
                          if nc.partition_id_tensor else None)
        in_names, out_names, out_avals = [], [], []
        out_shapes = []
        for alloc in nc.m.functions[0].allocations:
            if not isinstance(alloc, mybir.MemoryLocationSet):
                continue
            name = alloc.memorylocations[0].name
            if alloc.kind == "ExternalInput":
                if name != partition_name:
                    in_names.append(name)
            elif alloc.kind == "ExternalOutput":
                out_names.append(name)
                shape = tuple(alloc.tensor_shape)
                dtype = mybir.dt.np(alloc.dtype)
                out_avals.append(jax.core.ShapedArray(shape, dtype))
                out_shapes.append((shape, dtype))
        n_params = len(in_names)
        n_outs = len(out_names)
        bind_names = list(in_names) + list(out_names)
        if partition_name is not None:
            bind_names.append(partition_name)
        donate = tuple(range(n_params, n_params + n_outs))

        def _body(*args):
            operands = list(args)
            if partition_name is not None:
                operands.append(b2j.partition_id_tensor())
            outs = b2j._bass_exec_p.bind(
                *operands,
                out_avals=tuple(out_avals),
                in_names=tuple(bind_names),
                out_names=tuple(out_names),
                lowering_input_output_aliases=(),
                sim_require_finite=True,
                sim_require_nnan=True,
                nc=nc,
            )
            return tuple(outs)

        devices = jax.devices()[:NCORES]
        mesh = Mesh(np.asarray(devices), ("core",))
        in_specs = (PartitionSpec("core"),) * (n_params + n_outs)
        out_specs = (PartitionSpec("core"),) * n_outs
        sharded = jax.jit(
            shard_map(_body, mesh=mesh, in_specs=in_specs,
                      out_specs=out_specs, check_rep=False),
            donate_argnums=donate, keep_unused=True)
        _DISPATCH = (sharded, list(in_names), out_shapes)
    return _DISPATCH


_FAST_OK = True


def _run_device(vax):
    """Fast path: cached jitted shard_map dispatch.  Falls back to the
    stock run_bass_kernel_spmd if the cached-jit internals ever break."""
    global _FAST_OK
    if _FAST_OK:
        try:
            sharded, in_names, out_shapes = _get_dispatch()
            assert in_names == ["vx"], in_names
            concat_in = [np.ascontiguousarray(vax.reshape(NCORES * F, S * N))]
            concat_zeros = [np.zeros((NCORES * shp[0],) + shp[1:], dt)
                            for shp, dt in out_shapes]
            out_arrs = sharded(*concat_in, *concat_zeros)
            return np.asarray(out_arrs[0]).reshape(B, N)
        except Exception:
            _FAST_OK = False
    nc = _get_nc()
    in_maps = [{"vx": np.ascontiguousarray(vax[c])} for c in range(NCORES)]
    res = run_bass_kernel_spmd(nc, in_maps, list(range(NCORES)))
    return np.concatenate([res.results[c]["sdiag"] for c in range(NCORES)], 0)


def kernel(data: np.ndarray, lens: np.ndarray) -> np.ndarray:
    data = np.asarray(data, np.float32)
    lens = np.asarray(lens, np.int32)

    vax = _prep_inputs(data)
    sdiag = _run_device(vax)

    L = np.clip(lens, 1, N).astype(np.int64)
    pll = np.float32(1.0) + sdiag[np.arange(B), L - 1].astype(np.float32)
    R = (-GAMMA * np.log(np.maximum(pll, np.float32(1e-30)))).astype(np.float32)
    dists = (R / (np.float32(2.0) * L.astype(np.float32))).astype(np.float32)

    d = dists.reshape(NW_, STEP)
    dm = ((d[:, :, None] + d[:, None, :]) * np.float32(0.5)).astype(np.float32)
    g = NG_ + 1
    dmg = dm[:, :g, :g]
    neg = dm[:, :g, g:]
    scores = np.maximum(dmg[:, :, :, None] + MARGIN - neg[:, :, None, :],
                        np.float32(0.0))
    maxj = scores.max(axis=(2, 3)).astype(np.float32)
    sum_lks = maxj.sum(axis=1) * np.float32(g * NF_)
    nnz = (maxj != 0).astype(np.float32).sum(axis=1) * np.float32(g * NF_)
    lv = sum_lks / (nnz + np.float32(1.0))
    tril = np.tril(np.ones((g, g), bool), k=-1)
    only_pos = np.where(tril[None], dmg, np.float32(0.0)).sum(axis=(1, 2)) * (
        MODEL_LAMBDA / np.float32(NG_))
    loss = (lv + only_pos).sum() / np.float32(NW_)
    return np.float32(loss)
